# revision 5
# baseline (speedup 1.0000x reference)
"""ActiveShiftLayer Trainium2 kernel v3: single-pass direct 4-tap bilinear.

out[n,c,h,w] = bilinear sample of x[n,c, h+alpha_c, w+beta_c], zero outside.

Both floor shifts are baked into the HOST-STAGED input: channel c's plane is
placed at flat offset dst0_c = -(floor(alpha_c)*W + floor(beta_c)) in a
zero-padded [C, XLEN_H] fp16 tensor. On-chip the sample is then a UNIFORM
4-tap stencil at flat offsets {0, 1, W, W+1} with all-positive per-channel
weights w_ab (products of the fractional parts):

    OUT[k] = w00 X[k] + w01 X[k+1] + w10 X[k+W] + w11 X[k+W+1]

Per g=floor(beta) side, one tap PAIR is wrap-clean read flat (g=0: b=0 taps;
g=-1: b=1 taps) and the other pair wraps only at one column per row, where
its true contribution is zero. Channels are host-sorted g-major so at most
one 128-block mixes sides; per-channel ZERO WEIGHTS make every op legal over
all 128 partitions (no partition slicing).

Pieces (rows x W) are engine-assigned by pattern (ASL_PAT2 pure block /
ASL_PAT2M mixed block), cycled per piece:
  'p': PE 4 accumulating diag-matmul taps (chunk-major, 512) -> PSUM, drained
       by Scalar ACT / Vector copy (ASL_C2PAT2); the flat dirty-pair wraps
       are undone by negated strided STTs per piece-pair (2 pure / 4 mixed).
  'v': Vector: TS + STT flat clean taps + dirty 2D STTs that skip the wrap
       column (2+2 ops pure, 4+4 mixed). No fixups.

Stores per piece-pair slab via SWDGE early / sync HWDGE late. PE warm-up
holds the HAM clock; ASL_TAILMM>0 appends dummy matmuls to hold the clock
through the semaphore teardown.

Measured on trn2 (8 cores, core 0 profiled): ~53.7-55.7 us HW exec
(median ~54.7 over repeated runs, +-2-3 us run-to-run variance) vs 65.6 us
for the previous separable 2-stage kernel; l2 rel err ~3.6e-4 (gate 2e-2).
Span anatomy: ~7.2 us fixed runtime preamble gating the first DMA issue,
~43 us DMA-saturated steady state (12.98 MB at ~300-320 GB/s/core is the
roofline; Tensor ~39 us, Vector ~35 us, Scalar ~26 us all fit under it),
~8.9 us fixed framework teardown (257 semaphore clears at half clock --
HAM throttles once the PE idles). Tunables via ASL_* env vars; defaults
are the tuned values and are what the harness runs.
"""

import os
import numpy as np

N, C, H, W = 32, 256, 56, 56
NCORES = 8
NSH = N // NCORES
P = 128
CB = C // P
HW = H * W              # 3136
XLEN = HW + 2 * W       # 3248 = covers X3[P, 58, W] view reads
XST = HW + W + 2        # 3194 = staged/loaded content; tail memset to XLEN
BIG = os.environ.get("ASL_BIG", "1") == "1"  # 28-row pieces, single chain
PSLEN = 2048 if BIG else 1024  # PSUM banks per piece tile (4 / 2)
OFFS = (0, 1, W, W + 1)

PAT = os.environ.get("ASL_PAT2", "pv" if os.environ.get("ASL_BIG", "1") == "1" else "pvpv")
PATM = os.environ.get("ASL_PAT2M", "p")      # mixed-block piece pattern
C2PAT = os.environ.get("ASL_C2PAT2", "s")    # drain engine per 'p' piece
TAILMM = int(os.environ.get("ASL_TAILMM", "0"))
WARMMM = int(os.environ.get("ASL_WARMMM", "14"))
ILV = int(os.environ.get("ASL_ILV", "1" if BIG else "2"))
LQALT = os.environ.get("ASL_LQALT", "0") == "1"  # early loads on 2 queues

_CACHE = {}


def _build_nc(cfg):
    # cfg: per-block g-side: 0 (pure g=0), -1 (pure g=-1), 2 (mixed)
    gside = cfg
    import concourse.bacc as bacc
    import concourse.mybir as mybir
    import concourse.tile as tile

    f16 = mybir.dt.float16
    f32 = mybir.dt.float32
    mult = mybir.AluOpType.mult
    add = mybir.AluOpType.add
    act_copy = mybir.ActivationFunctionType.Copy

    nc = bacc.Bacc()
    xs = nc.dram_tensor("xs", [NSH, C, XST], f16, kind="ExternalInput")
    # wv16[cb] cols (zeroed where inapplicable by g-side):
    #  0 cA: clean0 (g=0, tap +0)    1 cB: clean0 (g=-1, tap +1)
    #  2 cC: clean1 (g=0, tap +W)    3 cD: clean1 (g=-1, tap +W+1)
    #  4 d0R,5 d1R: dirty a=0/1 (g=0, write [0:W-1], read X3[r+a, 1:W])
    #  6 d0L,7 d1L: dirty a=0/1 (g=-1, write [1:W], read X3[r+a, 1:W])
    #  8 -d0R,9 -d1R: fixup col W-1   10 -d0L,11 -d1L: fixup col 0
    #  12..15: w00 w01 w10 w11 (for the on-chip diag build)
    # both blocks side by side: [P, CB*16]
    wv16 = nc.dram_tensor("wv16", [P, CB * 16], f32, kind="ExternalInput")
    ys = nc.dram_tensor("ys", [NSH, C, H, W], f16, kind="ExternalOutput")

    with tile.TileContext(nc) as tc:
        import contextlib
        with tc.tile_pool(name="wp", bufs=1) as wp, \
             tc.tile_pool(name="xp", bufs=5) as xpool, \
             tc.tile_pool(name="op", bufs=4) as opool, \
             tc.tile_pool(name="ps0", bufs=2, space="PSUM") as ppool0, \
             (contextlib.nullcontext(ppool0) if BIG else
              tc.tile_pool(name="ps1", bufs=2, space="PSUM")) as ppool1:
            ppools = (ppool0, ppool1)

            wdt = []
            wvt = []

            # hoist the lazy ACT_TABLE_LOAD into the preamble
            sc = wp.tile([P, 2], f32, tag="sc")
            nc.gpsimd.memset(sc[:], 0.0)
            nc.scalar.activation(sc[:], sc[:], act_copy)
            # PE warm-up raises the HAM clock gate during the load head
            wu = wp.tile([P, 2 * P], f16, tag="wu")
            nc.gpsimd.memset(wu[:].bitcast(f32), 0.0)
            PSW = ppool0.tile([P, PSLEN], f32, tag="ps")
            for _ in range(WARMMM):
                nc.tensor.matmul(PSW[:, 0:256], wu[:, 0:P], wu[:, 0:2 * P],
                                 start=True, stop=True)

            # identity mask for building diag weight matrices on-chip
            MSK = wp.tile([P, P], f16, tag="msk")
            nc.gpsimd.memset(MSK[:], 1.0)
            nc.gpsimd.affine_select(
                out=MSK[:], in_=MSK[:], compare_op=mybir.AluOpType.is_equal,
                fill=0.0, base=0, pattern=[[-1, P]], channel_multiplier=1)

            def load_wv_all():
                v = wp.tile([P, CB * 16], f32, tag="wv")
                nc.sync.dma_start(v[:], wv16[:, :])
                for cb in range(CB):
                    wvt.append(v[:, cb * 16:(cb + 1) * 16])

            def build_wd(cb):
                t = wp.tile([P, 4 * P], f16, tag=f"wd{cb}")
                for tt in range(4):
                    nc.scalar.activation(t[:, tt * P:(tt + 1) * P], MSK[:],
                                         act_copy,
                                         scale=wvt[cb][:, 12 + tt:13 + tt])

                wdt.append(t)

            tiles = [(n, cb) for n in range(NSH) for cb in range(CB)]
            NT = len(tiles)

            def bounds(idx):
                if BIG:
                    if idx == 0:
                        return [0, 7, 14, 28, 42, 56]
                    if idx == 1:
                        return [0, 14, 28, 56]
                    if idx == NT - 1:
                        return [0, 14, 28, 42, 49, 56]
                    if idx == NT - 2:
                        return [0, 14, 28, 42, 56]
                    return [0, 28, 56]
                if idx == 0:
                    return [0, 4, 9, 16, 28, 42, 56]
                if idx == NT - 1:
                    return [0, 14, 28, 42, 49, 56]
                return [0, 14, 28, 42, 56]

            xtiles = {}

            def issue_load(idx):
                ln, lcb = tiles[idx]
                X = xpool.tile([P, XLEN], f16, tag="X")
                nc.gpsimd.memset(X[:, XST:XLEN].bitcast(f32), 0.0)
                xrow = xs[ln, lcb * P:(lcb + 1) * P, :]
                b = bounds(idx)
                if idx < 2:
                    cuts = [min((r + 2) * W + W + 2, XST)
                            for r in b[1:-1]] + [XST]
                elif idx < 4:
                    mid = b[len(b) // 2]
                    cuts = [min((mid + 2) * W + W + 2, XST), XST]
                else:
                    cuts = [XST]
                leng = (nc.gpsimd
                        if LQALT and idx % 2 == 1 and idx < 4 else nc.sync)
                c0 = 0
                for c1 in cuts:
                    if c1 > c0:
                        leng.dma_start(X[:, c0:c1], xrow[:, c0:c1])
                    c0 = c1
                xtiles[idx] = X

            pcnt = [0, 0]  # per-block piece counter for PAT/PATM
            c2cnt = [0]    # drain pattern counter

            def tile_gen(tidx):
                n, cb = tiles[tidx]
                gs = gside[cb]
                pat = PATM if gs == 2 else PAT
                ppool = ppools[tidx % 2]
                wvc = wvt[cb]
                cs = slice(cb * P, (cb + 1) * P)
                if tidx + 4 < NT:
                    issue_load(tidx + 4)
                X = xtiles.pop(tidx)
                X3 = X[:, 0:58 * W].rearrange("p (h w) -> p h w", w=W)
                OUT = opool.tile([P, HW], f16)
                O3 = OUT[:, 0:HW].rearrange("p (h w) -> p h w", w=W)

                def emit_p(p0, PZ):
                    PSA = ppool.tile([P, PSLEN], f32, tag="ps")
                    for c0 in range(0, PZ, 512):
                        cn = min(512, PZ - c0)
                        for t in range(4):
                            o = p0 + c0 + OFFS[t]
                            nc.tensor.matmul(
                                PSA[:, c0:c0 + cn],
                                wdt[cb][:, t * P:(t + 1) * P],
                                X[:, o:o + cn],
                                start=(t == 0), stop=(t == 3))
                    if C2PAT[c2cnt[0] % len(C2PAT)] == 's':
                        nc.scalar.activation(OUT[:, p0:p0 + PZ],
                                             PSA[:, 0:PZ], act_copy)
                    else:
                        nc.vector.tensor_copy(OUT[:, p0:p0 + PZ],
                                              PSA[:, 0:PZ])
                    c2cnt[0] += 1

                def fixup_p(rr0, rr1):
                    # undo flat dirty-pair wraps on 'p' rows [rr0, rr1)
                    if gs in (0, 2):   # col W-1, read X3[r+a+1, 0]
                        for a in (0, 1):
                            nc.vector.scalar_tensor_tensor(
                                O3[:, rr0:rr1, W - 1],
                                X3[:, rr0 + a + 1:rr1 + a + 1, 0],
                                wvc[:, 8 + a:9 + a],
                                O3[:, rr0:rr1, W - 1], op0=mult, op1=add)
                    if gs in (-1, 2):  # col 0, read X3[r+a, 0]
                        for a in (0, 1):
                            nc.vector.scalar_tensor_tensor(
                                O3[:, rr0:rr1, 0],
                                X3[:, rr0 + a:rr1 + a, 0],
                                wvc[:, 10 + a:11 + a],
                                O3[:, rr0:rr1, 0], op0=mult, op1=add)

                def emit_v(p0, PZ, rr0, rr1):
                    first = True

                    def acc(dst, src, wcol):
                        nonlocal first
                        if first:
                            nc.vector.tensor_scalar_mul(dst, src, wcol)
                            first = False
                        else:
                            nc.vector.scalar_tensor_tensor(
                                dst, src, wcol, dst, op0=mult, op1=add)

                    o = OUT[:, p0:p0 + PZ]
                    if gs in (0, 2):
                        acc(o, X[:, p0:p0 + PZ], wvc[:, 0:1])
                        acc(o, X[:, p0 + W:p0 + W + PZ], wvc[:, 2:3])
                    if gs in (-1, 2):
                        acc(o, X[:, p0 + 1:p0 + 1 + PZ], wvc[:, 1:2])
                        acc(o, X[:, p0 + W + 1:p0 + W + 1 + PZ], wvc[:, 3:4])
                    if gs in (0, 2):
                        for a in (0, 1):
                            nc.vector.scalar_tensor_tensor(
                                O3[:, rr0:rr1, 0:W - 1],
                                X3[:, rr0 + a:rr1 + a, 1:W],
                                wvc[:, 4 + a:5 + a],
                                O3[:, rr0:rr1, 0:W - 1], op0=mult, op1=add)
                    if gs in (-1, 2):
                        for a in (0, 1):
                            nc.vector.scalar_tensor_tensor(
                                O3[:, rr0:rr1, 1:W],
                                X3[:, rr0 + a:rr1 + a, 1:W],
                                wvc[:, 6 + a:7 + a],
                                O3[:, rr0:rr1, 1:W], op0=mult, op1=add)

                tb = bounds(tidx)
                pieces = list(zip(tb[:-1], tb[1:]))

                eng = nc.gpsimd if tidx % 2 == 0 and tidx < NT - 1 \
                    else nc.sync

                def store(s0, s1):
                    eng.dma_start(
                        ys[n, cs, s0:s1, :],
                        OUT[:, s0 * W:s1 * W].rearrange(
                            "p (h w) -> p h w", w=W))

                for rr0, rr1 in pieces:
                    c = pat[pcnt[cb] % len(pat)]
                    pcnt[cb] += 1
                    if c == 'p':
                        emit_p(rr0 * W, (rr1 - rr0) * W)
                        fixup_p(rr0, rr1)
                    else:
                        emit_v(rr0 * W, (rr1 - rr0) * W, rr0, rr1)
                    store(rr0, rr1)
                    yield
                yield

            load_wv_all()
            build_wd(0)
            build_wd(1)
            issue_load(0)
            issue_load(1)
            issue_load(2)
            issue_load(3)

            from collections import deque
            active = deque([tile_gen(i) for i in range(min(ILV, NT))])
            next_tile = len(active)
            while active:
                g = active.popleft()
                try:
                    next(g)
                    active.append(g)
                except StopIteration:
                    if next_tile < NT:
                        active.append(tile_gen(next_tile))
                        next_tile += 1

            # optional: hold the clock through teardown
            for _ in range(TAILMM):
                nc.tensor.matmul(PSW[:, 0:64], wu[:, 0:P], wu[:, 0:64],
                                 start=True, stop=True)
    nc.finalize()
    return nc


def _host_prep(sp):
    """Sort channels g-major; build weights, staging placement, block sides."""
    alpha = sp[:, 0].astype(np.float64)
    beta = sp[:, 1].astype(np.float64)
    f = np.floor(alpha)
    g = np.floor(beta)
    assert np.all((f == -1) | (f == 0)), "alpha outside [-1,1) unsupported"
    assert np.all((g == -1) | (g == 0)), "beta outside [-1,1) unsupported"
    perm = np.lexsort((f, g))  # g-major, f-minor
    fs = f[perm].astype(np.int32)
    gs = g[perm].astype(np.int32)
    ta = (alpha[perm] - fs)
    tb = (beta[perm] - gs)

    w00 = (1 - ta) * (1 - tb)
    w01 = (1 - ta) * tb
    w10 = ta * (1 - tb)
    w11 = ta * tb

    is0 = gs == 0
    clean0 = np.where(is0, w00, w01)
    clean1 = np.where(is0, w10, w11)
    dirty0 = np.where(is0, w01, w00)
    dirty1 = np.where(is0, w11, w10)
    z = np.zeros(C)
    cA = np.where(is0, clean0, z)
    cB = np.where(is0, z, clean0)
    cC = np.where(is0, clean1, z)
    cD = np.where(is0, z, clean1)
    d0R = np.where(is0, dirty0, z)
    d1R = np.where(is0, dirty1, z)
    d0L = np.where(is0, z, dirty0)
    d1L = np.where(is0, z, dirty1)

    wv16 = np.stack([cA, cB, cC, cD, d0R, d1R, d0L, d1L,
                     -d0R, -d1R, -d0L, -d1L, w00, w01, w10, w11],
                    axis=1).astype(np.float32).reshape(CB, P, 16)
    wv16 = np.ascontiguousarray(
        np.concatenate([wv16[cb] for cb in range(CB)], axis=1))

    dst0 = (-(fs * W + gs)).astype(np.int64)  # in {0, 1, W, W+1}
    gside = []
    for cb in range(CB):
        gseg = gs[cb * P:(cb + 1) * P]
        gside.append(2 if (gseg != gseg[0]).any() else int(gseg[0]))
    return perm, tuple(gside), dst0, np.ascontiguousarray(wv16)


def _stage_input(x, perm, dst0):
    """fp16 staging with per-channel flat shift baked in: [N, C, XLEN]."""
    xp = x[:, perm].astype(np.float16).reshape(N, C, HW)
    xsrt = np.zeros((N, C, XST), np.float16)
    for d in np.unique(dst0):
        m = dst0 == d
        xsrt[:, m, d:d + HW] = xp[:, m]
    return xsrt


def _install_trace_shim():
    import sys
    import types

    try:
        from antenv.axon_hooks import get_axon_ntff_profile_hook  # noqa: F401
    except ImportError:
        from trn_agent_boot.trn_boot import _ntff_profile_via_ctypes

        hook = _ntff_profile_via_ctypes("/opt/axon/libaxon_pjrt.so")
        mod = types.ModuleType("antenv.axon_hooks")
        mod.get_axon_ntff_profile_hook = lambda: hook
        mod.set_axon_ntff_profile_hook = lambda h: None
        import antenv

        sys.modules["antenv.axon_hooks"] = mod
        antenv.axon_hooks = mod

    import concourse.bass_utils as bu

    bu.upload_artifacts = lambda tmpdir: tmpdir


def kernel(x, shift_param):
    from concourse.bass_utils import run_bass_kernel_spmd

    x = np.asarray(x)
    sp = np.asarray(shift_param, dtype=np.float32)
    assert x.shape == (N, C, H, W)

    perm, gside, dst0, wv16 = _host_prep(sp)
    xsrt = _stage_input(x, perm, dst0)

    key = ("nc", gside)
    if key not in _CACHE:
        _CACHE[key] = _build_nc(gside)
    nc = _CACHE[key]

    in_maps = [{"xs": xsrt[i * NSH:(i + 1) * NSH], "wv16": wv16}
               for i in range(NCORES)]
    trace = os.environ.get("ASL_TRACE") == "1"
    if trace:
        _install_trace_shim()
    res = run_bass_kernel_spmd(nc, in_maps, list(range(NCORES)), trace=trace)
    if trace:
        print(f"HW exec time: {res.exec_time_ns} ns")
        _CACHE["last_result"] = res
    ys = np.concatenate([r["ys"] for r in res.results], axis=0)
    out = np.empty((N, C, H, W), np.float32)
    out[:, perm] = ys.astype(np.float32)
    return out


# revision 6
# speedup vs baseline: 1.1162x; 1.1162x over previous
"""ActiveShiftLayer Trainium2 kernel v3: single-pass direct 4-tap bilinear.

out[n,c,h,w] = bilinear sample of x[n,c, h+alpha_c, w+beta_c], zero outside.

Both floor shifts are baked into the HOST-STAGED input: channel c's plane is
placed at flat offset dst0_c = -(floor(alpha_c)*W + floor(beta_c)) in a
zero-padded [C, XLEN_H] fp16 tensor. On-chip the sample is then a UNIFORM
4-tap stencil at flat offsets {0, 1, W, W+1} with all-positive per-channel
weights w_ab (products of the fractional parts):

    OUT[k] = w00 X[k] + w01 X[k+1] + w10 X[k+W] + w11 X[k+W+1]

Per g=floor(beta) side, one tap PAIR is wrap-clean read flat (g=0: b=0 taps;
g=-1: b=1 taps) and the other pair wraps only at one column per row, where
its true contribution is zero. Channels are host-sorted g-major so at most
one 128-block mixes sides; per-channel ZERO WEIGHTS make every op legal over
all 128 partitions (no partition slicing).

Pieces (rows x W) are engine-assigned by pattern (ASL_PAT2 pure block /
ASL_PAT2M mixed block), cycled per piece:
  'p': PE 4 accumulating diag-matmul taps (chunk-major, 512) -> PSUM, drained
       by Scalar ACT / Vector copy (ASL_C2PAT2); the flat dirty-pair wraps
       are undone by negated strided STTs per piece-pair (2 pure / 4 mixed).
  'v': Vector: TS + STT flat clean taps + dirty 2D STTs that skip the wrap
       column (2+2 ops pure, 4+4 mixed). No fixups.

Stores per piece-pair slab via SWDGE early / sync HWDGE late. PE warm-up
holds the HAM clock; ASL_TAILMM>0 appends dummy matmuls to hold the clock
through the semaphore teardown.

Measured on trn2 (8 cores, core 0 profiled): ~53.7-55.7 us HW exec
(median ~54.7 over repeated runs, +-2-3 us run-to-run variance) vs 65.6 us
for the previous separable 2-stage kernel; l2 rel err ~3.6e-4 (gate 2e-2).
Span anatomy: ~7.2 us fixed runtime preamble gating the first DMA issue,
~43 us DMA-saturated steady state (12.98 MB at ~300-320 GB/s/core is the
roofline; Tensor ~39 us, Vector ~35 us, Scalar ~26 us all fit under it),
~8.9 us fixed framework teardown (257 semaphore clears at half clock --
HAM throttles once the PE idles). Tunables via ASL_* env vars; defaults
are the tuned values and are what the harness runs.
"""

import os
import numpy as np

N, C, H, W = 32, 256, 56, 56
NCORES = 8
NSH = N // NCORES
P = 128
CB = C // P
HW = H * W              # 3136
XLEN = HW + 2 * W       # 3248 = covers X3[P, 58, W] view reads
XST = HW + W + 2        # 3194 = staged/loaded content; tail memset to XLEN
BIG = os.environ.get("ASL_BIG", "1") == "1"  # 28-row pieces, single chain
PSLEN = 2048 if BIG else 1024  # PSUM banks per piece tile (4 / 2)
OFFS = (0, 1, W, W + 1)

PAT = os.environ.get("ASL_PAT2", "pv" if os.environ.get("ASL_BIG", "1") == "1" else "pvpv")
PATM = os.environ.get("ASL_PAT2M", "p")      # mixed-block piece pattern
C2PAT = os.environ.get("ASL_C2PAT2", "s")    # drain engine per 'p' piece
TAILMM = int(os.environ.get("ASL_TAILMM", "0"))
WARMMM = int(os.environ.get("ASL_WARMMM", "14"))
ILV = int(os.environ.get("ASL_ILV", "1" if BIG else "2"))
LQALT = os.environ.get("ASL_LQALT", "0") == "1"  # early loads on 2 queues

_CACHE = {}


def _build_nc(cfg):
    # cfg: per-block g-side: 0 (pure g=0), -1 (pure g=-1), 2 (mixed)
    gside = cfg
    import concourse.bacc as bacc
    import concourse.mybir as mybir
    import concourse.tile as tile

    f16 = mybir.dt.float16
    f32 = mybir.dt.float32
    mult = mybir.AluOpType.mult
    add = mybir.AluOpType.add
    act_copy = mybir.ActivationFunctionType.Copy

    nc = bacc.Bacc()
    xs = nc.dram_tensor("xs", [NSH, C, XST], f16, kind="ExternalInput")
    # wv16[cb] cols (zeroed where inapplicable by g-side):
    #  0 cA: clean0 (g=0, tap +0)    1 cB: clean0 (g=-1, tap +1)
    #  2 cC: clean1 (g=0, tap +W)    3 cD: clean1 (g=-1, tap +W+1)
    #  4 d0R,5 d1R: dirty a=0/1 (g=0, write [0:W-1], read X3[r+a, 1:W])
    #  6 d0L,7 d1L: dirty a=0/1 (g=-1, write [1:W], read X3[r+a, 1:W])
    #  8 -d0R,9 -d1R: fixup col W-1   10 -d0L,11 -d1L: fixup col 0
    #  12..15: w00 w01 w10 w11 (for the on-chip diag build)
    # both blocks side by side: [P, CB*16]
    wv16 = nc.dram_tensor("wv16", [P, CB * 16], f32, kind="ExternalInput")
    ys = nc.dram_tensor("ys", [NSH, C, H, W], f16, kind="ExternalOutput")

    with tile.TileContext(nc) as tc:
        import contextlib
        with tc.tile_pool(name="wp", bufs=1) as wp, \
             tc.tile_pool(name="xp", bufs=5) as xpool, \
             tc.tile_pool(name="op", bufs=4) as opool, \
             tc.tile_pool(name="ps0", bufs=2, space="PSUM") as ppool0, \
             (contextlib.nullcontext(ppool0) if BIG else
              tc.tile_pool(name="ps1", bufs=2, space="PSUM")) as ppool1:
            ppools = (ppool0, ppool1)

            wdt = []
            wvt = []

            # hoist the lazy ACT_TABLE_LOAD into the preamble
            sc = wp.tile([P, 2], f32, tag="sc")
            nc.gpsimd.memset(sc[:], 0.0)
            nc.scalar.activation(sc[:], sc[:], act_copy)
            # PE warm-up raises the HAM clock gate during the load head
            wu = wp.tile([P, 2 * P], f16, tag="wu")
            nc.gpsimd.memset(wu[:].bitcast(f32), 0.0)
            PSW = ppool0.tile([P, PSLEN], f32, tag="ps")
            for _ in range(WARMMM):
                nc.tensor.matmul(PSW[:, 0:256], wu[:, 0:P], wu[:, 0:2 * P],
                                 start=True, stop=True)

            # identity mask for building diag weight matrices on-chip
            MSK = wp.tile([P, P], f16, tag="msk")
            nc.gpsimd.memset(MSK[:], 1.0)
            nc.gpsimd.affine_select(
                out=MSK[:], in_=MSK[:], compare_op=mybir.AluOpType.is_equal,
                fill=0.0, base=0, pattern=[[-1, P]], channel_multiplier=1)

            def load_wv_all():
                v = wp.tile([P, CB * 16], f32, tag="wv")
                nc.sync.dma_start(v[:], wv16[:, :])
                for cb in range(CB):
                    wvt.append(v[:, cb * 16:(cb + 1) * 16])

            def build_wd(cb):
                t = wp.tile([P, 4 * P], f16, tag=f"wd{cb}")
                for tt in range(4):
                    nc.scalar.activation(t[:, tt * P:(tt + 1) * P], MSK[:],
                                         act_copy,
                                         scale=wvt[cb][:, 12 + tt:13 + tt])

                wdt.append(t)

            tiles = [(n, cb) for n in range(NSH) for cb in range(CB)]
            NT = len(tiles)

            def bounds(idx):
                if BIG:
                    if idx == 0:
                        return [0, 7, 14, 28, 42, 56]
                    if idx == 1:
                        return [0, 14, 28, 56]
                    if idx == NT - 1:
                        return [0, 28, 42, 49, 56]
                    return [0, 28, 56]
                if idx == 0:
                    return [0, 4, 9, 16, 28, 42, 56]
                if idx == NT - 1:
                    return [0, 14, 28, 42, 49, 56]
                return [0, 14, 28, 42, 56]

            xtiles = {}

            def issue_load(idx):
                ln, lcb = tiles[idx]
                X = xpool.tile([P, XLEN], f16, tag="X")
                nc.gpsimd.memset(X[:, XST:XLEN].bitcast(f32), 0.0)
                xrow = xs[ln, lcb * P:(lcb + 1) * P, :]
                b = bounds(idx)
                if idx < 2:
                    cuts = [min((r + 2) * W + W + 2, XST)
                            for r in b[1:-1]] + [XST]
                elif idx < 4:
                    mid = b[len(b) // 2]
                    cuts = [min((mid + 2) * W + W + 2, XST), XST]
                else:
                    cuts = [XST]
                leng = (nc.gpsimd
                        if LQALT and idx % 2 == 1 and idx < 4 else nc.sync)
                c0 = 0
                for c1 in cuts:
                    if c1 > c0:
                        leng.dma_start(X[:, c0:c1], xrow[:, c0:c1])
                    c0 = c1
                xtiles[idx] = X

            pcnt = [0, 0]  # per-block piece counter for PAT/PATM
            c2cnt = [0]    # drain pattern counter

            def tile_gen(tidx):
                n, cb = tiles[tidx]
                gs = gside[cb]
                pat = PATM if gs == 2 else PAT
                ppool = ppools[tidx % 2]
                wvc = wvt[cb]
                cs = slice(cb * P, (cb + 1) * P)
                if tidx + 4 < NT:
                    issue_load(tidx + 4)
                X = xtiles.pop(tidx)
                X3 = X[:, 0:58 * W].rearrange("p (h w) -> p h w", w=W)
                OUT = opool.tile([P, HW], f16)
                O3 = OUT[:, 0:HW].rearrange("p (h w) -> p h w", w=W)

                def emit_p(p0, PZ):
                    PSA = ppool.tile([P, PSLEN], f32, tag="ps")
                    for c0 in range(0, PZ, 512):
                        cn = min(512, PZ - c0)
                        for t in range(4):
                            o = p0 + c0 + OFFS[t]
                            nc.tensor.matmul(
                                PSA[:, c0:c0 + cn],
                                wdt[cb][:, t * P:(t + 1) * P],
                                X[:, o:o + cn],
                                start=(t == 0), stop=(t == 3))
                    if C2PAT[c2cnt[0] % len(C2PAT)] == 's':
                        nc.scalar.activation(OUT[:, p0:p0 + PZ],
                                             PSA[:, 0:PZ], act_copy)
                    else:
                        nc.vector.tensor_copy(OUT[:, p0:p0 + PZ],
                                              PSA[:, 0:PZ])
                    c2cnt[0] += 1

                def fixup_p(rr0, rr1):
                    # undo flat dirty-pair wraps on 'p' rows [rr0, rr1)
                    if gs in (0, 2):   # col W-1, read X3[r+a+1, 0]
                        for a in (0, 1):
                            nc.vector.scalar_tensor_tensor(
                                O3[:, rr0:rr1, W - 1],
                                X3[:, rr0 + a + 1:rr1 + a + 1, 0],
                                wvc[:, 8 + a:9 + a],
                                O3[:, rr0:rr1, W - 1], op0=mult, op1=add)
                    if gs in (-1, 2):  # col 0, read X3[r+a, 0]
                        for a in (0, 1):
                            nc.vector.scalar_tensor_tensor(
                                O3[:, rr0:rr1, 0],
                                X3[:, rr0 + a:rr1 + a, 0],
                                wvc[:, 10 + a:11 + a],
                                O3[:, rr0:rr1, 0], op0=mult, op1=add)

                def emit_v(p0, PZ, rr0, rr1):
                    first = True

                    def acc(dst, src, wcol):
                        nonlocal first
                        if first:
                            nc.vector.tensor_scalar_mul(dst, src, wcol)
                            first = False
                        else:
                            nc.vector.scalar_tensor_tensor(
                                dst, src, wcol, dst, op0=mult, op1=add)

                    o = OUT[:, p0:p0 + PZ]
                    if gs in (0, 2):
                        acc(o, X[:, p0:p0 + PZ], wvc[:, 0:1])
                        acc(o, X[:, p0 + W:p0 + W + PZ], wvc[:, 2:3])
                    if gs in (-1, 2):
                        acc(o, X[:, p0 + 1:p0 + 1 + PZ], wvc[:, 1:2])
                        acc(o, X[:, p0 + W + 1:p0 + W + 1 + PZ], wvc[:, 3:4])
                    if gs in (0, 2):
                        for a in (0, 1):
                            nc.vector.scalar_tensor_tensor(
                                O3[:, rr0:rr1, 0:W - 1],
                                X3[:, rr0 + a:rr1 + a, 1:W],
                                wvc[:, 4 + a:5 + a],
                                O3[:, rr0:rr1, 0:W - 1], op0=mult, op1=add)
                    if gs in (-1, 2):
                        for a in (0, 1):
                            nc.vector.scalar_tensor_tensor(
                                O3[:, rr0:rr1, 1:W],
                                X3[:, rr0 + a:rr1 + a, 1:W],
                                wvc[:, 6 + a:7 + a],
                                O3[:, rr0:rr1, 1:W], op0=mult, op1=add)

                tb = bounds(tidx)
                pieces = list(zip(tb[:-1], tb[1:]))

                eng = nc.gpsimd if tidx % 2 == 0 and tidx < NT - 1 \
                    else nc.sync

                def store(s0, s1):
                    eng.dma_start(
                        ys[n, cs, s0:s1, :],
                        OUT[:, s0 * W:s1 * W].rearrange(
                            "p (h w) -> p h w", w=W))

                for rr0, rr1 in pieces:
                    c = pat[pcnt[cb] % len(pat)]
                    pcnt[cb] += 1
                    if c == 'p':
                        emit_p(rr0 * W, (rr1 - rr0) * W)
                        fixup_p(rr0, rr1)
                    else:
                        emit_v(rr0 * W, (rr1 - rr0) * W, rr0, rr1)
                    store(rr0, rr1)
                    yield
                yield

            load_wv_all()
            build_wd(0)
            build_wd(1)
            issue_load(0)
            issue_load(1)
            issue_load(2)
            issue_load(3)

            from collections import deque
            active = deque([tile_gen(i) for i in range(min(ILV, NT))])
            next_tile = len(active)
            while active:
                g = active.popleft()
                try:
                    next(g)
                    active.append(g)
                except StopIteration:
                    if next_tile < NT:
                        active.append(tile_gen(next_tile))
                        next_tile += 1

            # optional: hold the clock through teardown
            for _ in range(TAILMM):
                nc.tensor.matmul(PSW[:, 0:64], wu[:, 0:P], wu[:, 0:64],
                                 start=True, stop=True)
    nc.finalize()
    return nc


def _host_prep(sp):
    """Sort channels g-major; build weights, staging placement, block sides."""
    alpha = sp[:, 0].astype(np.float64)
    beta = sp[:, 1].astype(np.float64)
    f = np.floor(alpha)
    g = np.floor(beta)
    assert np.all((f == -1) | (f == 0)), "alpha outside [-1,1) unsupported"
    assert np.all((g == -1) | (g == 0)), "beta outside [-1,1) unsupported"
    perm = np.lexsort((f, g))  # g-major, f-minor
    fs = f[perm].astype(np.int32)
    gs = g[perm].astype(np.int32)
    ta = (alpha[perm] - fs)
    tb = (beta[perm] - gs)

    w00 = (1 - ta) * (1 - tb)
    w01 = (1 - ta) * tb
    w10 = ta * (1 - tb)
    w11 = ta * tb

    is0 = gs == 0
    clean0 = np.where(is0, w00, w01)
    clean1 = np.where(is0, w10, w11)
    dirty0 = np.where(is0, w01, w00)
    dirty1 = np.where(is0, w11, w10)
    z = np.zeros(C)
    cA = np.where(is0, clean0, z)
    cB = np.where(is0, z, clean0)
    cC = np.where(is0, clean1, z)
    cD = np.where(is0, z, clean1)
    d0R = np.where(is0, dirty0, z)
    d1R = np.where(is0, dirty1, z)
    d0L = np.where(is0, z, dirty0)
    d1L = np.where(is0, z, dirty1)

    wv16 = np.stack([cA, cB, cC, cD, d0R, d1R, d0L, d1L,
                     -d0R, -d1R, -d0L, -d1L, w00, w01, w10, w11],
                    axis=1).astype(np.float32).reshape(CB, P, 16)
    wv16 = np.ascontiguousarray(
        np.concatenate([wv16[cb] for cb in range(CB)], axis=1))

    dst0 = (-(fs * W + gs)).astype(np.int64)  # in {0, 1, W, W+1}
    gside = []
    for cb in range(CB):
        gseg = gs[cb * P:(cb + 1) * P]
        gside.append(2 if (gseg != gseg[0]).any() else int(gseg[0]))
    return perm, tuple(gside), dst0, np.ascontiguousarray(wv16)


def _stage_input(x, perm, dst0):
    """fp16 staging with per-channel flat shift baked in: [N, C, XLEN]."""
    xp = x[:, perm].astype(np.float16).reshape(N, C, HW)
    xsrt = np.zeros((N, C, XST), np.float16)
    for d in np.unique(dst0):
        m = dst0 == d
        xsrt[:, m, d:d + HW] = xp[:, m]
    return xsrt


def _install_trace_shim():
    import sys
    import types

    try:
        from antenv.axon_hooks import get_axon_ntff_profile_hook  # noqa: F401
    except ImportError:
        from trn_agent_boot.trn_boot import _ntff_profile_via_ctypes

        hook = _ntff_profile_via_ctypes("/opt/axon/libaxon_pjrt.so")
        mod = types.ModuleType("antenv.axon_hooks")
        mod.get_axon_ntff_profile_hook = lambda: hook
        mod.set_axon_ntff_profile_hook = lambda h: None
        import antenv

        sys.modules["antenv.axon_hooks"] = mod
        antenv.axon_hooks = mod

    import concourse.bass_utils as bu

    bu.upload_artifacts = lambda tmpdir: tmpdir


def kernel(x, shift_param):
    from concourse.bass_utils import run_bass_kernel_spmd

    x = np.asarray(x)
    sp = np.asarray(shift_param, dtype=np.float32)
    assert x.shape == (N, C, H, W)

    perm, gside, dst0, wv16 = _host_prep(sp)
    xsrt = _stage_input(x, perm, dst0)

    key = ("nc", gside)
    if key not in _CACHE:
        _CACHE[key] = _build_nc(gside)
    nc = _CACHE[key]

    in_maps = [{"xs": xsrt[i * NSH:(i + 1) * NSH], "wv16": wv16}
               for i in range(NCORES)]
    trace = os.environ.get("ASL_TRACE") == "1"
    if trace:
        _install_trace_shim()
    res = run_bass_kernel_spmd(nc, in_maps, list(range(NCORES)), trace=trace)
    if trace:
        print(f"HW exec time: {res.exec_time_ns} ns")
        _CACHE["last_result"] = res
    ys = np.concatenate([r["ys"] for r in res.results], axis=0)
    out = np.empty((N, C, H, W), np.float32)
    out[:, perm] = ys.astype(np.float32)
    return out
